# revision 1
# baseline (speedup 1.0000x reference)
"""ArcFace margin-injection kernel for one TRN2 chip (8 NeuronCores).

out = logits * 64 everywhere, except at (row, label) for rows with a valid
label, where out = cos(arccos(clip(x)) + 0.5) * 64.

Strategy: shard the batch dim across the 8 cores (512 rows each; every
sample's target column then lives entirely on its own core). On device,
stream [128, 6250] tiles: corr = (iota == target_col_p) * (64*delta_p) via a
2x-mode tensor_scalar, then out = (x * 64) + corr via one fused
scalar_tensor_tensor, DMA out. The margin values themselves (4096 scalars)
are precomputed on host in the sharding step; all full-tensor work is on
device. Memory-bound: ~205 MB HBM traffic per core.
"""
import sys

for _p in ("/opt/trn_rl_repo", "/opt/pypackages"):
    if _p not in sys.path:
        sys.path.append(_p)

import numpy as np

B, C = 4096, 50000
NCORES = 8
ROWS = B // NCORES           # 512 rows per core
P = 128                      # SBUF partitions
F = 6250                     # free-dim chunk
NRT = ROWS // P              # 4 row tiles per core
NCK = C // F                 # 8 column chunks
NT = NRT * NCK               # 32 tiles per core
S = 64.0
MARGIN = 0.5

_STATE = {}


def _build_nc(tin_bufs=3, corr_bufs=2, out_bufs=2):
    import concourse.tile as tile
    from concourse import bacc, mybir

    f32 = mybir.dt.float32
    i16 = mybir.dt.int16

    nc = bacc.Bacc(None, target_bir_lowering=False)
    x = nc.declare_dram_parameter("x", [ROWS, C], f32, isOutput=False)
    iota = nc.declare_dram_parameter("iota", [P, F], i16, isOutput=False)
    cols = nc.declare_dram_parameter("cols", [P, NT], f32, isOutput=False)
    sdelta = nc.declare_dram_parameter("sdelta", [P, NT], f32, isOutput=False)
    out = nc.declare_dram_parameter("out", [ROWS, C], f32, isOutput=True)

    with tile.TileContext(nc) as tc:
        with (
            tc.tile_pool(name="const", bufs=1) as const_pool,
            tc.tile_pool(name="tin", bufs=tin_bufs) as tin_pool,
            tc.tile_pool(name="corr", bufs=corr_bufs) as corr_pool,
            tc.tile_pool(name="outp", bufs=out_bufs) as out_pool,
        ):
            iota_sb = const_pool.tile([P, F], i16)
            nc.sync.dma_start(iota_sb[:], iota[:])
            cols_sb = const_pool.tile([P, NT], f32)
            nc.sync.dma_start(cols_sb[:], cols[:])
            sdelta_sb = const_pool.tile([P, NT], f32)
            nc.sync.dma_start(sdelta_sb[:], sdelta[:])

            for rt in range(NRT):
                for ck in range(NCK):
                    t = rt * NCK + ck
                    tin = tin_pool.tile([P, F], f32)
                    nc.sync.dma_start(
                        tin[:], x[rt * P:(rt + 1) * P, ck * F:(ck + 1) * F]
                    )
                    corr = corr_pool.tile([P, F], f32)
                    nc.vector.tensor_scalar(
                        corr[:],
                        iota_sb[:],
                        cols_sb[:, t:t + 1],
                        sdelta_sb[:, t:t + 1],
                        mybir.AluOpType.is_equal,
                        mybir.AluOpType.mult,
                    )
                    o = out_pool.tile([P, F], f32)
                    nc.vector.scalar_tensor_tensor(
                        o[:],
                        tin[:],
                        S,
                        corr[:],
                        mybir.AluOpType.mult,
                        mybir.AluOpType.add,
                    )
                    nc.scalar.dma_start(
                        out[rt * P:(rt + 1) * P, ck * F:(ck + 1) * F], o[:]
                    )
    nc.compile()
    return nc


def _get_nc():
    if "nc" not in _STATE:
        _STATE["nc"] = _build_nc()
    return _STATE["nc"]


def _host_prep(logits, labels):
    """Shard inputs + precompute per-core scatter scalars (O(B) host work)."""
    labels = labels.astype(np.int64)
    valid = labels != -1
    safe = np.where(valid, labels, 0)
    x_t = logits[np.arange(B), safe].astype(np.float64)
    corrected = np.cos(np.arccos(np.clip(x_t, -1.0, 1.0)) + MARGIN)
    sdelta_full = np.where(valid, S * (corrected - x_t), 0.0).astype(np.float32)

    ck = safe // F
    local = (safe - ck * F).astype(np.float32)

    # layout [core, p, rt, ck] -> reshape to [core, P, NT] with t = rt*NCK+ck
    cols_arr = np.full((NCORES, P, NRT, NCK), -1.0, np.float32)
    sdelta_arr = np.zeros((NCORES, P, NRT, NCK), np.float32)
    r = np.arange(B)
    core_i = r // ROWS
    rt_i = (r % ROWS) // P
    p_i = r % P
    vr = valid
    cols_arr[core_i[vr], p_i[vr], rt_i[vr], ck[vr]] = local[vr]
    sdelta_arr[core_i[vr], p_i[vr], rt_i[vr], ck[vr]] = sdelta_full[vr]
    cols_arr = cols_arr.reshape(NCORES, P, NT)
    sdelta_arr = sdelta_arr.reshape(NCORES, P, NT)

    iota_np = np.ascontiguousarray(
        np.broadcast_to(np.arange(F, dtype=np.int16), (P, F))
    )
    in_maps = []
    for c in range(NCORES):
        in_maps.append(
            {
                "x": logits[c * ROWS:(c + 1) * ROWS],
                "iota": iota_np,
                "cols": cols_arr[c],
                "sdelta": sdelta_arr[c],
            }
        )
    return in_maps


def _run(logits, labels, trace=False, trace_kwargs=None):
    from concourse.bass_utils import run_bass_kernel_spmd

    logits = np.ascontiguousarray(np.asarray(logits, dtype=np.float32))
    labels = np.asarray(labels)
    in_maps = _host_prep(logits, labels)
    nc = _get_nc()
    res = run_bass_kernel_spmd(
        nc,
        in_maps,
        core_ids=list(range(NCORES)),
        trace=trace,
        **(trace_kwargs or {}),
    )
    out = np.concatenate([r["out"] for r in res.results], axis=0)
    return out, res


def kernel(**inputs):
    out, _ = _run(inputs["logits"], inputs["labels"], trace=False)
    return out
